# revision 9
# baseline (speedup 1.0000x reference)
"""MoE feed-forward (top-2 of 8 experts) on 8 TRN2 NeuronCores.

Strategy (expert-parallel, per the sharding hint):
  - Host: gate in fp64 (logits -> softmax -> top-2), pack each expert's
    routed tokens into a fixed-capacity buffer, one expert per core.
  - Core e: H^T = gelu(W1[e]^T x^T + b1[e]) (bf16 GEMM), spill H^T to
    DRAM in bf16, then Y = w_tok * (H W2[e]) (all-bf16 GEMM2, d-dim
    split in two double-buffered halves), Y back to host.
  - Host: scatter-add the 8 per-expert outputs into the dense result.

All matmuls are bf16 (walrus rejects mixed f32r/bf16 operands and true
fp32 runs at 1/4 rate); accumulation is fp32 in PSUM.
"""

import os

if os.environ.get("JAX_PLATFORMS") == "cpu":
    # The bass kernel executes through the axon PJRT platform; a cpu-only
    # pin would leave no NeuronCores visible.
    os.environ["JAX_PLATFORMS"] = ""

import numpy as np
import ml_dtypes

P = 128
D = 2048
F = 5632
E = 8
TOP_K = 2
N_CORES = 8


def _gate_host(flat, gate_w, gate_b):
    """fp64 gating: returns per-token top-k expert ids and softmax scores."""
    logits = flat.astype(np.float64) @ gate_w.astype(np.float64) + gate_b.astype(
        np.float64
    )
    m = logits.max(axis=-1, keepdims=True)
    e = np.exp(logits - m)
    s = e / e.sum(axis=-1, keepdims=True)
    # stable argsort of -s == lax.top_k tie-breaking (lowest index first)
    order = np.argsort(-s, axis=-1, kind="stable")
    top_i = order[:, :TOP_K]
    return top_i, s


def _build_program(cap):
    import concourse.bass as bass
    import concourse.mybir as mybir
    import concourse.tile as tile

    f32 = mybir.dt.float32
    bf16 = mybir.dt.bfloat16

    KD = D // P  # 16 k-tiles over D
    FT = F // P  # 44 f-tiles
    FQ = FT // 4  # 11 f-quad groups
    TT = cap // P  # token tiles for GEMM2
    DH = D // 2  # 1024, d-half for GEMM2
    W1B = 256  # W1 f-block columns per streamed tile
    # GEMM1 token tiles: fp32r moving operand needs N>=256 for full rate
    tok_tiles = [512] * (cap // 512)
    if cap % 512:
        tok_tiles.append(cap % 512)
    assert cap % 256 == 0 and all(t >= 256 for t in tok_tiles)

    nc = bass.Bass()
    xT = nc.dram_tensor("xT", [D, cap], bf16, kind="ExternalInput")
    w1 = nc.dram_tensor("w1", [D, F], bf16, kind="ExternalInput")
    w2 = nc.dram_tensor("w2", [F, D], bf16, kind="ExternalInput")
    b1 = nc.dram_tensor("b1", [F], f32, kind="ExternalInput")
    wt = nc.dram_tensor("wt", [cap], f32, kind="ExternalInput")
    y = nc.dram_tensor("y", [cap, D], f32, kind="ExternalOutput")
    hT = nc.dram_tensor("hT", [F, cap], bf16)

    xT_t = xT.rearrange("(ko p) n -> p ko n", p=P)
    w1_t = w1.rearrange("(ko p) f -> p ko f", p=P)
    w2_t = w2.rearrange("(fo p) d -> p fo d", p=P)
    hT_t = hT.rearrange("(fo p) n -> p fo n", p=P)

    with tile.TileContext(nc) as tc:
        with (
            tc.tile_pool(name="const", bufs=1) as constp,
            tc.tile_pool(name="w2pool", bufs=1) as w2pool,
        ):
            b1_sb = constp.tile([P, FT], f32)
            nc.sync.dma_start(b1_sb[:], b1.rearrange("(o p) -> p o", p=P))
            wt_sb = constp.tile([P, TT], f32)
            nc.sync.dma_start(wt_sb[:], wt.rearrange("(o p) -> p o", p=P))

            # First w2 half sits below xT on the stack so its load overlaps
            # phase A and phase B can start with zero weight-load bubble.
            w2_h0 = w2pool.tile([P, FT, DH], bf16, name="w2_h0")
            nc.sync.dma_start(w2_h0[:], w2_t[:, :, 0:DH])

            # ---- Phase A: hT = gelu(w1.T @ x.T + b1), spilled to DRAM ----
            with (
                tc.tile_pool(name="xpool", bufs=1) as xpool,
                tc.tile_pool(name="w1pool", bufs=3) as w1pool,
                tc.tile_pool(name="hpool", bufs=4) as hpool,
                tc.tile_pool(name="psA", bufs=4, space="PSUM") as psA,
            ):
                xT_sb = xpool.tile([P, KD, cap], bf16)
                nc.sync.dma_start(xT_sb[:], xT_t[:])
                for fb in range(F // W1B):
                    w1_sb = w1pool.tile([P, KD, W1B], bf16, tag="w1sb")
                    nc.sync.dma_start(
                        w1_sb[:], w1_t[:, :, fb * W1B : (fb + 1) * W1B]
                    )
                    for fl in range(W1B // P):
                        ft = fb * (W1B // P) + fl
                        tok0 = 0
                        for tokN in tok_tiles:
                            ps = psA.tile([P, 512], f32, tag="psA")
                            for k in range(KD):
                                nc.tensor.matmul(
                                    ps[:, :tokN],
                                    lhsT=w1_sb[:, k, fl * P : (fl + 1) * P],
                                    rhs=xT_sb[:, k, tok0 : tok0 + tokN],
                                    start=(k == 0),
                                    stop=(k == KD - 1),
                                )
                            hsb = hpool.tile([P, 512], bf16, tag="hsb")
                            nc.scalar.activation(
                                hsb[:, :tokN],
                                ps[:, :tokN],
                                mybir.ActivationFunctionType.Gelu,
                                bias=b1_sb[:, ft : ft + 1],
                            )
                            nc.sync.dma_start(
                                hT[ft * P : (ft + 1) * P, tok0 : tok0 + tokN],
                                hsb[:, :tokN],
                            )
                            tok0 += tokN

            # ---- Phase B: y = wt * (hT.T @ w2), two d-halves ----
            with (
                tc.tile_pool(name="w2pool1", bufs=1) as w2pool1,
                tc.tile_pool(name="hbpool", bufs=3) as hbpool,
                tc.tile_pool(name="ypool", bufs=3) as ypool,
                tc.tile_pool(name="psB", bufs=2, space="PSUM") as psB,
            ):
                w2_h1 = w2pool1.tile([P, FT, DH], bf16, name="w2_h1")
                nc.sync.dma_start(w2_h1[:], w2_t[:, :, DH : 2 * DH])
                for dh, w2_sb in ((0, w2_h0), (1, w2_h1)):
                    for tt in range(TT):
                        ps = psB.tile([P, DH], f32, tag="psB")
                        for fq in range(FQ):
                            hb = hbpool.tile([P, 4, P], bf16, tag="hb")
                            nc.sync.dma_start(
                                hb[:],
                                hT_t[:, fq * 4 : (fq + 1) * 4, tt * P : (tt + 1) * P],
                            )
                            for sub in range(4):
                                f = fq * 4 + sub
                                for half in range(2):
                                    nc.tensor.matmul(
                                        ps[:, half * 512 : (half + 1) * 512],
                                        lhsT=hb[:, sub, :],
                                        rhs=w2_sb[:, f, half * 512 : (half + 1) * 512],
                                        start=(f == 0),
                                        stop=(f == FT - 1),
                                    )
                        yt = ypool.tile([P, DH], f32, tag="yt")
                        nc.vector.tensor_scalar_mul(
                            yt[:], ps[:], wt_sb[:, tt : tt + 1]
                        )
                        nc.sync.dma_start(
                            y[tt * P : (tt + 1) * P, dh * DH : (dh + 1) * DH],
                            yt[:],
                        )

    _split_multi_waits(nc)
    return nc


def _split_multi_waits(nc):
    """The walrus build in this container rejects >1 sync-wait command per
    instruction; hoist extras onto single-wait NOPs just before it."""
    import bass_rust
    import concourse.mybir as mybir

    ctr = 0
    for blk in nc.m.functions[0].blocks:
        insts = blk.instructions
        i = 0
        while i < len(insts):
            inst = insts[i]
            si = inst.sync_info
            if si is None:
                i += 1
                continue
            waits = list(si.on_wait)
            if len(waits) <= 1:
                i += 1
                continue
            for w in waits[:-1]:
                ctr += 1
                nop = bass_rust.InstNoOp(name=f"waitsplit_{ctr}")
                nop.engine = inst.engine
                nop.sync_info = mybir.SyncInfo(on_wait=[w], on_update=[])
                insts.insert(i, nop)
                i += 1
            inst.sync_info = mybir.SyncInfo(
                on_wait=[waits[-1]], on_update=list(si.on_update)
            )
            i += 1


_CACHE = {}


def _get_program(cap):
    if cap not in _CACHE:
        _CACHE[cap] = _build_program(cap)
    return _CACHE[cap]


def prepare(x, gate_w, gate_b, W1, b1, W2, b2):
    """Host routing + per-core input packing. Returns (in_maps, idx, cap,
    top_i, scores, flat_shape)."""
    x = np.asarray(x, np.float32)
    B, S, Dx = x.shape
    assert (Dx, W1.shape[2], gate_b.shape[0]) == (D, F, E)
    T = B * S
    flat = x.reshape(T, D)

    top_i, scores = _gate_host(flat, np.asarray(gate_w), np.asarray(gate_b))

    idx = []
    wts = []
    for e in range(E):
        sel = np.where((top_i == e).any(axis=1))[0]
        idx.append(sel)
        wts.append(scores[sel, e].astype(np.float32))
    max_load = max(len(s) for s in idx)
    cap = max(512, -(-max_load // 256) * 256)

    w1b = np.ascontiguousarray(np.asarray(W1, np.float32)).astype(ml_dtypes.bfloat16)
    w2b = np.ascontiguousarray(np.asarray(W2, np.float32)).astype(ml_dtypes.bfloat16)
    b1 = np.asarray(b1, np.float32)

    in_maps = []
    for e in range(E):
        n_e = len(idx[e])
        xTe = np.zeros((D, cap), ml_dtypes.bfloat16)
        xTe[:, :n_e] = flat[idx[e]].T.astype(ml_dtypes.bfloat16)
        wte = np.zeros((cap,), np.float32)
        wte[:n_e] = wts[e]
        in_maps.append(
            {
                "xT": xTe,
                "w1": w1b[e],
                "w2": w2b[e],
                "b1": b1[e],
                "wt": wte,
            }
        )
    return in_maps, idx, cap, top_i, scores, (B, S, T)


def combine(results, idx, top_i, scores, b2, shape):
    B, S, T = shape
    b2 = np.asarray(b2, np.float32)
    out = np.zeros((T, D), np.float32)
    for e in range(E):
        n_e = len(idx[e])
        out[idx[e]] += results[e]["y"][:n_e]
    if np.any(b2):
        w_dense = np.zeros((T, E), np.float32)
        for k in range(TOP_K):
            w_dense[np.arange(T), top_i[:, k]] += scores[
                np.arange(T), top_i[:, k]
            ].astype(np.float32)
        out += w_dense @ b2
    return out.reshape(B, S, D)


def kernel(x, gate_w, gate_b, W1, b1, W2, b2):
    from concourse.bass_utils import run_bass_kernel_spmd

    in_maps, idx, cap, top_i, scores, shape = prepare(
        x, gate_w, gate_b, W1, b1, W2, b2
    )
    nc = _get_program(cap)
    res = run_bass_kernel_spmd(nc, in_maps, list(range(N_CORES)))
    return combine(res.results, idx, top_i, scores, b2, shape)


# revision 21
# speedup vs baseline: 1.2039x; 1.2039x over previous
"""MoE feed-forward (top-2 of 8 experts) on 8 TRN2 NeuronCores.

Strategy (expert-parallel, per the sharding hint):
  - Host: gate in fp64 (logits -> softmax -> top-2), pack each expert's
    routed tokens into a fixed-capacity buffer, one expert per core.
  - Core e: H^T = gelu(W1[e]^T x^T + b1[e]) (bf16 GEMM), spill H^T to
    DRAM in bf16, then Y = w_tok * (H W2[e]) (all-bf16 GEMM2, d-dim
    split in two double-buffered halves), Y back to host.
  - Host: scatter-add the 8 per-expert outputs into the dense result.

All matmuls are bf16 (walrus rejects mixed f32r/bf16 operands and true
fp32 runs at 1/4 rate); accumulation is fp32 in PSUM.
"""

import os

if os.environ.get("JAX_PLATFORMS") == "cpu":
    # The bass kernel executes through the axon PJRT platform; a cpu-only
    # pin would leave no NeuronCores visible.
    os.environ["JAX_PLATFORMS"] = ""

import numpy as np
import ml_dtypes

P = 128
D = 2048
F = 5632
E = 8
TOP_K = 2
N_CORES = 8


def _gate_host(flat, gate_w, gate_b):
    """fp64 gating: returns per-token top-k expert ids and softmax scores."""
    logits = flat.astype(np.float64) @ gate_w.astype(np.float64) + gate_b.astype(
        np.float64
    )
    m = logits.max(axis=-1, keepdims=True)
    e = np.exp(logits - m)
    s = e / e.sum(axis=-1, keepdims=True)
    # stable argsort of -s == lax.top_k tie-breaking (lowest index first)
    order = np.argsort(-s, axis=-1, kind="stable")
    top_i = order[:, :TOP_K]
    return top_i, s


def _build_program(cap):
    import concourse.bass as bass
    import concourse.mybir as mybir
    import concourse.tile as tile

    f32 = mybir.dt.float32
    bf16 = mybir.dt.bfloat16

    KD = D // P  # 16 k-tiles over D
    FT = F // P  # 44 f-tiles
    FQ = FT // 4  # 11 f-quad groups
    TT = cap // P  # token tiles for GEMM2
    DH = D // 2  # 1024, d-half for GEMM2
    W1B = 256  # W1 f-block columns per streamed tile
    # GEMM1 token tiles (bf16 moving operand: full rate at any N)
    tok_tiles = [512] * (cap // 512)
    if cap % 512:
        tok_tiles.append(cap % 512)
    assert cap % P == 0

    nc = bass.Bass()
    xT = nc.dram_tensor("xT", [D, cap], bf16, kind="ExternalInput")
    w1 = nc.dram_tensor("w1", [D, F], bf16, kind="ExternalInput")
    w2 = nc.dram_tensor("w2", [F, D], bf16, kind="ExternalInput")
    b1 = nc.dram_tensor("b1", [F], f32, kind="ExternalInput")
    wt = nc.dram_tensor("wt", [cap], f32, kind="ExternalInput")
    y = nc.dram_tensor("y", [cap, D], f32, kind="ExternalOutput")
    hT = nc.dram_tensor("hT", [F, cap], bf16)

    xT_t = xT.rearrange("(ko p) n -> p ko n", p=P)
    w1_t = w1.rearrange("(ko p) f -> p ko f", p=P)
    w2_t = w2.rearrange("(fo p) d -> p fo d", p=P)
    hT_t = hT.rearrange("(fo p) n -> p fo n", p=P)

    with tile.TileContext(nc) as tc:
        with (
            tc.tile_pool(name="const", bufs=1) as constp,
            tc.tile_pool(name="w2pool", bufs=1) as w2pool,
        ):
            # First w2 half sits below xT on the stack; its chunk loads are
            # sprinkled through the phase-A loop so they never delay the
            # critical-path xT/w1 loads in the DMA queues.
            w2_h0 = w2pool.tile([P, FT, DH], bf16, name="w2_h0")

            # ---- Phase A: hT = gelu(w1.T @ x.T + b1), spilled to DRAM ----
            with (
                tc.tile_pool(name="xpool", bufs=1) as xpool,
                tc.tile_pool(name="w1pool", bufs=3) as w1pool,
                tc.tile_pool(name="hpool", bufs=4) as hpool,
                tc.tile_pool(name="psA", bufs=4, space="PSUM") as psA,
            ):
                xT_sb = xpool.tile([P, KD, cap], bf16)
                # first token block's slices lead the queue; the first w1
                # block follows; only then the rest of xT and the consts
                first_tok = min(tok_tiles[0], cap)
                for t0 in range(0, first_tok, 256):
                    t1 = min(t0 + 256, first_tok)
                    nc.sync.dma_start(xT_sb[:, :, t0:t1], xT_t[:, :, t0:t1])
                w1_first = w1pool.tile([P, KD, W1B], bf16, tag="w1sb")
                nc.sync.dma_start(w1_first[:], w1_t[:, :, 0:W1B])
                for t0 in range(first_tok, cap, 256):
                    t1 = min(t0 + 256, cap)
                    nc.sync.dma_start(xT_sb[:, :, t0:t1], xT_t[:, :, t0:t1])
                b1_sb = constp.tile([P, FT], f32)
                nc.sync.dma_start(b1_sb[:], b1.rearrange("(o p) -> p o", p=P))
                wt_sb = constp.tile([P, TT], f32)
                nc.sync.dma_start(wt_sb[:], wt.rearrange("(o p) -> p o", p=P))
                for fb in range(F // W1B):
                    if fb == 0:
                        w1_sb = w1_first
                    else:
                        w1_sb = w1pool.tile([P, KD, W1B], bf16, tag="w1sb")
                        nc.sync.dma_start(
                            w1_sb[:], w1_t[:, :, fb * W1B : (fb + 1) * W1B]
                        )
                    # ride a w2_h0 chunk behind each w1 block load (22
                    # iterations x 2 fo-slices covers all 44)
                    fo0, fo1 = fb * 2, min(fb * 2 + 2, FT)
                    nc.sync.dma_start(
                        w2_h0[:, fo0:fo1, :], w2_t[:, fo0:fo1, 0:DH]
                    )
                    for fl in range(W1B // P):
                        ft = fb * (W1B // P) + fl
                        tok0 = 0
                        for tokN in tok_tiles:
                            ps = psA.tile([P, 512], f32, tag="psA")
                            for k in range(KD):
                                nc.tensor.matmul(
                                    ps[:, :tokN],
                                    lhsT=w1_sb[:, k, fl * P : (fl + 1) * P],
                                    rhs=xT_sb[:, k, tok0 : tok0 + tokN],
                                    start=(k == 0),
                                    stop=(k == KD - 1),
                                )
                            hsb = hpool.tile([P, 512], bf16, tag="hsb")
                            nc.scalar.activation(
                                hsb[:, :tokN],
                                ps[:, :tokN],
                                mybir.ActivationFunctionType.Gelu,
                                bias=b1_sb[:, ft : ft + 1],
                            )
                            nc.gpsimd.dma_start(
                                hT[ft * P : (ft + 1) * P, tok0 : tok0 + tokN],
                                hsb[:, :tokN],
                            )
                            tok0 += tokN

            # ---- Phase B: y = wt * (hT.T @ w2), two d-halves ----
            with (
                tc.tile_pool(name="w2pool1", bufs=1) as w2pool1,
                tc.tile_pool(name="hbpool", bufs=6) as hbpool,
                tc.tile_pool(name="ypool", bufs=3) as ypool,
                tc.tile_pool(name="psB", bufs=2, space="PSUM") as psB,
            ):
                w2_h1 = w2pool1.tile([P, FT, DH], bf16, name="w2_h1")
                for dh, w2_sb in ((0, w2_h0), (1, w2_h1)):
                    for tt in range(TT):
                        if dh == 0:
                            # sprinkle w2_h1 chunk loads behind dh0 compute
                            n_ch = (FT + TT - 1) // TT
                            fo0 = tt * n_ch
                            fo1 = min(fo0 + n_ch, FT)
                            if fo0 < FT:
                                nc.sync.dma_start(
                                    w2_h1[:, fo0:fo1, :],
                                    w2_t[:, fo0:fo1, DH : 2 * DH],
                                )
                        ps = psB.tile([P, DH], f32, tag="psB")
                        for fq in range(FQ):
                            hb = hbpool.tile([P, 4, P], bf16, tag="hb")
                            nc.sync.dma_start(
                                hb[:],
                                hT_t[:, fq * 4 : (fq + 1) * 4, tt * P : (tt + 1) * P],
                            )
                            for sub in range(4):
                                f = fq * 4 + sub
                                for half in range(2):
                                    nc.tensor.matmul(
                                        ps[:, half * 512 : (half + 1) * 512],
                                        lhsT=hb[:, sub, :],
                                        rhs=w2_sb[:, f, half * 512 : (half + 1) * 512],
                                        start=(f == 0),
                                        stop=(f == FT - 1),
                                    )
                        yt = ypool.tile([P, DH], f32, tag="yt")
                        nc.vector.tensor_scalar_mul(
                            yt[:], ps[:], wt_sb[:, tt : tt + 1]
                        )
                        nc.gpsimd.dma_start(
                            y[tt * P : (tt + 1) * P, dh * DH : (dh + 1) * DH],
                            yt[:],
                        )

    _split_multi_waits(nc)
    return nc


def _split_multi_waits(nc):
    """The walrus build in this container rejects >1 sync-wait command per
    instruction; hoist extras onto single-wait NOPs just before it."""
    import bass_rust
    import concourse.mybir as mybir

    ctr = 0
    for blk in nc.m.functions[0].blocks:
        insts = blk.instructions
        i = 0
        while i < len(insts):
            inst = insts[i]
            si = inst.sync_info
            if si is None:
                i += 1
                continue
            waits = list(si.on_wait)
            if len(waits) <= 1:
                i += 1
                continue
            for w in waits[:-1]:
                ctr += 1
                nop = bass_rust.InstNoOp(name=f"waitsplit_{ctr}")
                nop.engine = inst.engine
                nop.sync_info = mybir.SyncInfo(on_wait=[w], on_update=[])
                insts.insert(i, nop)
                i += 1
            inst.sync_info = mybir.SyncInfo(
                on_wait=[waits[-1]], on_update=list(si.on_update)
            )
            i += 1


_CACHE = {}


def _get_program(cap):
    if cap not in _CACHE:
        _CACHE[cap] = _build_program(cap)
    return _CACHE[cap]


def prepare(x, gate_w, gate_b, W1, b1, W2, b2):
    """Host routing + per-core input packing. Returns (in_maps, idx, cap,
    top_i, scores, flat_shape)."""
    x = np.asarray(x, np.float32)
    B, S, Dx = x.shape
    assert (Dx, W1.shape[2], gate_b.shape[0]) == (D, F, E)
    T = B * S
    flat = x.reshape(T, D)

    top_i, scores = _gate_host(flat, np.asarray(gate_w), np.asarray(gate_b))

    idx = []
    wts = []
    for e in range(E):
        sel = np.where((top_i == e).any(axis=1))[0]
        idx.append(sel)
        wts.append(scores[sel, e].astype(np.float32))
    max_load = max(len(s) for s in idx)
    cap = max(512, -(-max_load // P) * P)

    w1b = np.ascontiguousarray(np.asarray(W1, np.float32)).astype(ml_dtypes.bfloat16)
    w2b = np.ascontiguousarray(np.asarray(W2, np.float32)).astype(ml_dtypes.bfloat16)
    b1 = np.asarray(b1, np.float32)

    in_maps = []
    for e in range(E):
        n_e = len(idx[e])
        xTe = np.zeros((D, cap), ml_dtypes.bfloat16)
        xTe[:, :n_e] = flat[idx[e]].T.astype(ml_dtypes.bfloat16)
        wte = np.zeros((cap,), np.float32)
        wte[:n_e] = wts[e]
        in_maps.append(
            {
                "xT": xTe,
                "w1": w1b[e],
                "w2": w2b[e],
                "b1": b1[e],
                "wt": wte,
            }
        )
    return in_maps, idx, cap, top_i, scores, (B, S, T)


def combine(results, idx, top_i, scores, b2, shape):
    B, S, T = shape
    b2 = np.asarray(b2, np.float32)
    out = np.zeros((T, D), np.float32)
    for e in range(E):
        n_e = len(idx[e])
        out[idx[e]] += results[e]["y"][:n_e]
    if np.any(b2):
        w_dense = np.zeros((T, E), np.float32)
        for k in range(TOP_K):
            w_dense[np.arange(T), top_i[:, k]] += scores[
                np.arange(T), top_i[:, k]
            ].astype(np.float32)
        out += w_dense @ b2
    return out.reshape(B, S, D)


def kernel(x, gate_w, gate_b, W1, b1, W2, b2):
    from concourse.bass_utils import run_bass_kernel_spmd

    in_maps, idx, cap, top_i, scores, shape = prepare(
        x, gate_w, gate_b, W1, b1, W2, b2
    )
    nc = _get_program(cap)
    res = run_bass_kernel_spmd(nc, in_maps, list(range(N_CORES)))
    return combine(res.results, idx, top_i, scores, b2, shape)


# revision 26
# speedup vs baseline: 1.8074x; 1.5013x over previous
"""MoE feed-forward (top-2 of 8 experts) on 8 TRN2 NeuronCores.

Strategy (expert-parallel, per the sharding hint):
  - Host: gate in fp64 (logits -> softmax -> top-2), pack each expert's
    routed tokens into a fixed-capacity buffer, one expert per core.
  - Core e: H^T = gelu(W1[e]^T x^T + b1[e]) (bf16 GEMM), spill H^T to
    DRAM in bf16, then Y = w_tok * (H W2[e]) (all-bf16 GEMM2, d-dim
    split in two double-buffered halves), Y back to host.
  - Host: scatter-add the 8 per-expert outputs into the dense result.

All matmuls are bf16 (walrus rejects mixed f32r/bf16 operands and true
fp32 runs at 1/4 rate); accumulation is fp32 in PSUM.
"""

import os

if os.environ.get("JAX_PLATFORMS") == "cpu":
    # The bass kernel executes through the axon PJRT platform; a cpu-only
    # pin would leave no NeuronCores visible.
    os.environ["JAX_PLATFORMS"] = ""

import numpy as np
import ml_dtypes

P = 128
D = 2048
F = 5632
E = 8
TOP_K = 2
N_CORES = 8


def _gate_host(flat, gate_w, gate_b):
    """fp64 gating: returns per-token top-k expert ids and softmax scores."""
    logits = flat.astype(np.float64) @ gate_w.astype(np.float64) + gate_b.astype(
        np.float64
    )
    m = logits.max(axis=-1, keepdims=True)
    e = np.exp(logits - m)
    s = e / e.sum(axis=-1, keepdims=True)
    # stable argsort of -s == lax.top_k tie-breaking (lowest index first)
    order = np.argsort(-s, axis=-1, kind="stable")
    top_i = order[:, :TOP_K]
    return top_i, s


def _build_program(cap):
    import concourse.bass as bass
    import concourse.mybir as mybir
    import concourse.tile as tile

    f32 = mybir.dt.float32
    bf16 = mybir.dt.bfloat16

    KD = D // P  # 16 k-tiles over D
    FT = F // P  # 44 f-tiles
    FQ = FT // 4  # 11 f-quad groups
    TT = cap // P  # token tiles for GEMM2
    DH = D // 2  # 1024, d-half for GEMM2
    W1B = 256  # W1 f-block columns per streamed tile
    # GEMM1 token tiles (bf16 moving operand: full rate at any N)
    tok_tiles = [512] * (cap // 512)
    if cap % 512:
        tok_tiles.append(cap % 512)
    assert cap % P == 0

    nc = bass.Bass()
    xT = nc.dram_tensor("xT", [D, cap], bf16, kind="ExternalInput")
    w1 = nc.dram_tensor("w1", [D, F], bf16, kind="ExternalInput")
    w2 = nc.dram_tensor("w2", [F, D], bf16, kind="ExternalInput")
    b1 = nc.dram_tensor("b1", [F], f32, kind="ExternalInput")
    wt = nc.dram_tensor("wt", [cap], f32, kind="ExternalInput")
    y = nc.dram_tensor("y", [cap, D], f32, kind="ExternalOutput")
    hT = nc.dram_tensor("hT", [F, cap], bf16)

    xT_t = xT.rearrange("(ko p) n -> p ko n", p=P)
    w1_t = w1.rearrange("(ko p) f -> p ko f", p=P)
    w2_t = w2.rearrange("(fo p) d -> p fo d", p=P)
    hT_t = hT.rearrange("(fo p) n -> p fo n", p=P)

    with tile.TileContext(nc) as tc:
        with (
            tc.tile_pool(name="const", bufs=1) as constp,
            tc.tile_pool(name="w2pool", bufs=1) as w2pool,
        ):
            # First w2 half sits below xT on the stack; its chunk loads are
            # sprinkled through the phase-A loop so they never delay the
            # critical-path xT/w1 loads in the DMA queues.
            w2_h0 = w2pool.tile([P, FT, DH], bf16, name="w2_h0")

            # ---- Phase A: hT = gelu(w1.T @ x.T + b1), spilled to DRAM ----
            with (
                tc.tile_pool(name="xpool", bufs=1) as xpool,
                tc.tile_pool(name="w1pool", bufs=3) as w1pool,
                tc.tile_pool(name="hpool", bufs=4) as hpool,
                tc.tile_pool(name="psA", bufs=4, space="PSUM") as psA,
            ):
                xT_sb = xpool.tile([P, KD, cap], bf16)
                # first token block's slices lead the queue; the first w1
                # block follows; only then the rest of xT and the consts
                first_tok = min(tok_tiles[0], cap)
                nc.sync.dma_start(
                    xT_sb[:, :, 0:first_tok], xT_t[:, :, 0:first_tok]
                )
                w1_first = w1pool.tile([P, KD, W1B], bf16, tag="w1sb")
                nc.sync.dma_start(w1_first[:], w1_t[:, :, 0:W1B])
                for t0 in range(first_tok, cap, 256):
                    t1 = min(t0 + 256, cap)
                    nc.sync.dma_start(xT_sb[:, :, t0:t1], xT_t[:, :, t0:t1])
                b1_sb = constp.tile([P, FT], f32)
                nc.sync.dma_start(b1_sb[:], b1.rearrange("(o p) -> p o", p=P))
                wt_sb = constp.tile([P, TT], f32)
                nc.sync.dma_start(wt_sb[:], wt.rearrange("(o p) -> p o", p=P))
                for fb in range(F // W1B):
                    if fb == 0:
                        w1_sb = w1_first
                    else:
                        w1_sb = w1pool.tile([P, KD, W1B], bf16, tag="w1sb")
                        nc.sync.dma_start(
                            w1_sb[:], w1_t[:, :, fb * W1B : (fb + 1) * W1B]
                        )
                    # ride a w2_h0 chunk behind each w1 block load (22
                    # iterations x 2 fo-slices covers all 44)
                    fo0, fo1 = fb * 2, min(fb * 2 + 2, FT)
                    nc.sync.dma_start(
                        w2_h0[:, fo0:fo1, :], w2_t[:, fo0:fo1, 0:DH]
                    )
                    for fl in range(W1B // P):
                        ft = fb * (W1B // P) + fl
                        tok0 = 0
                        for tokN in tok_tiles:
                            ps = psA.tile([P, 512], f32, tag="psA")
                            for k in range(KD):
                                nc.tensor.matmul(
                                    ps[:, :tokN],
                                    lhsT=w1_sb[:, k, fl * P : (fl + 1) * P],
                                    rhs=xT_sb[:, k, tok0 : tok0 + tokN],
                                    start=(k == 0),
                                    stop=(k == KD - 1),
                                )
                            hsb = hpool.tile([P, 512], bf16, tag="hsb")
                            nc.scalar.activation(
                                hsb[:, :tokN],
                                ps[:, :tokN],
                                mybir.ActivationFunctionType.Gelu,
                                bias=b1_sb[:, ft : ft + 1],
                            )
                            nc.gpsimd.dma_start(
                                hT[ft * P : (ft + 1) * P, tok0 : tok0 + tokN],
                                hsb[:, :tokN],
                            )
                            tok0 += tokN

            # ---- Phase B: y = wt * (hT.T @ w2), two d-halves ----
            with (
                tc.tile_pool(name="w2pool1", bufs=1) as w2pool1,
                tc.tile_pool(name="hbpool", bufs=6) as hbpool,
                tc.tile_pool(name="ypool", bufs=3) as ypool,
                tc.tile_pool(name="psB", bufs=2, space="PSUM") as psB,
            ):
                w2_h1 = w2pool1.tile([P, FT, DH], bf16, name="w2_h1")
                for dh, w2_sb in ((0, w2_h0), (1, w2_h1)):
                    for tt in range(TT):
                        if dh == 0:
                            # sprinkle w2_h1 chunk loads behind dh0 compute
                            n_ch = (FT + TT - 1) // TT
                            fo0 = tt * n_ch
                            fo1 = min(fo0 + n_ch, FT)
                            if fo0 < FT:
                                nc.sync.dma_start(
                                    w2_h1[:, fo0:fo1, :],
                                    w2_t[:, fo0:fo1, DH : 2 * DH],
                                )
                        ps = psB.tile([P, DH], f32, tag="psB")
                        for fq in range(FQ):
                            hb = hbpool.tile([P, 4, P], bf16, tag="hb")
                            nc.sync.dma_start(
                                hb[:],
                                hT_t[:, fq * 4 : (fq + 1) * 4, tt * P : (tt + 1) * P],
                            )
                            for sub in range(4):
                                f = fq * 4 + sub
                                for half in range(2):
                                    nc.tensor.matmul(
                                        ps[:, half * 512 : (half + 1) * 512],
                                        lhsT=hb[:, sub, :],
                                        rhs=w2_sb[:, f, half * 512 : (half + 1) * 512],
                                        start=(f == 0),
                                        stop=(f == FT - 1),
                                    )
                        yt = ypool.tile([P, DH], f32, tag="yt")
                        nc.vector.tensor_scalar_mul(
                            yt[:], ps[:], wt_sb[:, tt : tt + 1]
                        )
                        nc.gpsimd.dma_start(
                            y[tt * P : (tt + 1) * P, dh * DH : (dh + 1) * DH],
                            yt[:],
                        )

    _split_multi_waits(nc)
    return nc


def _split_multi_waits(nc):
    """The walrus build in this container rejects >1 sync-wait command per
    instruction; hoist extras onto single-wait NOPs just before it."""
    import bass_rust
    import concourse.mybir as mybir

    ctr = 0
    for blk in nc.m.functions[0].blocks:
        insts = blk.instructions
        i = 0
        while i < len(insts):
            inst = insts[i]
            si = inst.sync_info
            if si is None:
                i += 1
                continue
            waits = list(si.on_wait)
            if len(waits) <= 1:
                i += 1
                continue
            for w in waits[:-1]:
                ctr += 1
                nop = bass_rust.InstNoOp(name=f"waitsplit_{ctr}")
                nop.engine = inst.engine
                nop.sync_info = mybir.SyncInfo(on_wait=[w], on_update=[])
                insts.insert(i, nop)
                i += 1
            inst.sync_info = mybir.SyncInfo(
                on_wait=[waits[-1]], on_update=list(si.on_update)
            )
            i += 1


_CACHE = {}


def _get_program(cap):
    if cap not in _CACHE:
        _CACHE[cap] = _build_program(cap)
    return _CACHE[cap]


_RUNNER_CACHE = {}


def _make_runner(nc, n_cores=N_CORES):
    """Persistent jitted shard_map over the bass NEFF (one jax.jit per
    program, reused across kernel() calls)."""
    import jax
    from jax.sharding import Mesh, PartitionSpec
    from jax.experimental.shard_map import shard_map
    import concourse.mybir as mybir
    from concourse import bass2jax
    from concourse.bass2jax import _bass_exec_p, partition_id_tensor

    bass2jax.install_neuronx_cc_hook()

    partition_name = nc.partition_id_tensor.name if nc.partition_id_tensor else None
    in_names, out_names, out_avals, zero_shapes = [], [], [], []
    for alloc in nc.m.functions[0].allocations:
        if not isinstance(alloc, mybir.MemoryLocationSet):
            continue
        name = alloc.memorylocations[0].name
        if alloc.kind == "ExternalInput":
            if name != partition_name:
                in_names.append(name)
        elif alloc.kind == "ExternalOutput":
            out_names.append(name)
            shape = tuple(alloc.tensor_shape)
            dtype = mybir.dt.np(alloc.dtype)
            out_avals.append(jax.core.ShapedArray(shape, dtype))
            zero_shapes.append((shape, dtype))
    n_params = len(in_names)
    n_outs = len(out_avals)
    in_names.extend(out_names)
    if partition_name is not None:
        in_names.append(partition_name)

    def _body(*args):
        operands = list(args)
        if partition_name is not None:
            operands.append(partition_id_tensor())
        outs = _bass_exec_p.bind(
            *operands,
            out_avals=tuple(out_avals),
            in_names=tuple(in_names),
            out_names=tuple(out_names),
            lowering_input_output_aliases=(),
            sim_require_finite=True,
            sim_require_nnan=True,
            nc=nc,
        )
        return tuple(outs)

    devices = jax.devices()[:n_cores]
    mesh = Mesh(np.asarray(devices), ("core",))
    in_specs = (PartitionSpec("core"),) * (n_params + n_outs)
    out_specs = (PartitionSpec("core"),) * len(out_names)
    donate = tuple(range(n_params, n_params + n_outs))
    sharded = jax.jit(
        shard_map(
            _body, mesh=mesh, in_specs=in_specs, out_specs=out_specs, check_rep=False
        ),
        donate_argnums=donate,
        keep_unused=True,
    )

    def run(in_maps):
        per_core = [
            [np.asarray(m[name]) for name in in_names[:n_params]] for m in in_maps
        ]
        concat_in = [
            np.concatenate([per_core[c][i] for c in range(n_cores)], axis=0)
            for i in range(n_params)
        ]
        concat_zeros = [
            np.zeros((n_cores * s[0], *s[1:]), dt) for s, dt in zero_shapes
        ]
        out_arrs = sharded(*concat_in, *concat_zeros)
        return [
            {
                name: np.asarray(out_arrs[i]).reshape(
                    n_cores, *out_avals[i].shape
                )[c]
                for i, name in enumerate(out_names)
            }
            for c in range(n_cores)
        ]

    return run


def _get_runner(cap):
    if cap not in _RUNNER_CACHE:
        _RUNNER_CACHE[cap] = _make_runner(_get_program(cap))
    return _RUNNER_CACHE[cap]


def prepare(x, gate_w, gate_b, W1, b1, W2, b2):
    """Host routing + per-core input packing. Returns (in_maps, idx, cap,
    top_i, scores, flat_shape)."""
    x = np.asarray(x, np.float32)
    B, S, Dx = x.shape
    assert (Dx, W1.shape[2], gate_b.shape[0]) == (D, F, E)
    T = B * S
    flat = x.reshape(T, D)

    top_i, scores = _gate_host(flat, np.asarray(gate_w), np.asarray(gate_b))

    idx = []
    wts = []
    for e in range(E):
        sel = np.where((top_i == e).any(axis=1))[0]
        idx.append(sel)
        wts.append(scores[sel, e].astype(np.float32))
    max_load = max(len(s) for s in idx)
    cap = max(512, -(-max_load // P) * P)

    w1b = np.ascontiguousarray(np.asarray(W1, np.float32)).astype(ml_dtypes.bfloat16)
    w2b = np.ascontiguousarray(np.asarray(W2, np.float32)).astype(ml_dtypes.bfloat16)
    b1 = np.asarray(b1, np.float32)

    in_maps = []
    for e in range(E):
        n_e = len(idx[e])
        xTe = np.zeros((D, cap), ml_dtypes.bfloat16)
        xTe[:, :n_e] = flat[idx[e]].T.astype(ml_dtypes.bfloat16)
        wte = np.zeros((cap,), np.float32)
        wte[:n_e] = wts[e]
        in_maps.append(
            {
                "xT": xTe,
                "w1": w1b[e],
                "w2": w2b[e],
                "b1": b1[e],
                "wt": wte,
            }
        )
    return in_maps, idx, cap, top_i, scores, (B, S, T)


def combine(results, idx, top_i, scores, b2, shape):
    B, S, T = shape
    b2 = np.asarray(b2, np.float32)
    out = np.zeros((T, D), np.float32)
    for e in range(E):
        n_e = len(idx[e])
        out[idx[e]] += results[e]["y"][:n_e]
    if np.any(b2):
        w_dense = np.zeros((T, E), np.float32)
        for k in range(TOP_K):
            w_dense[np.arange(T), top_i[:, k]] += scores[
                np.arange(T), top_i[:, k]
            ].astype(np.float32)
        out += w_dense @ b2
    return out.reshape(B, S, D)


def kernel(x, gate_w, gate_b, W1, b1, W2, b2):
    in_maps, idx, cap, top_i, scores, shape = prepare(
        x, gate_w, gate_b, W1, b1, W2, b2
    )
    results = _get_runner(cap)(in_maps)
    return combine(results, idx, top_i, scores, b2, shape)
